# revision 45
# baseline (speedup 1.0000x reference)
"""Trainium2 Bass kernel for nn_CATransformer1 (XCiT-style channel-attention block).

Sharding: data-parallel over batch. 16 images / 8 cores = 2 images per core.
Weights replicated; no collectives.

V2 design (bf16 everywhere):
  - x is DMAed once per image (bf16) and stays SBUF-resident across both
    passes; output written back as bf16 and widened on host.
  - All matmuls run in bf16 (full rate at any free-dim size on TRN2).
  - LN1/LN2 stats are computed in column form (per-pixel partitions) with
    F=1 ones-matmuls (nearly free on the PE), then transposed to row form
    for the rank-1 mean terms and row-broadcasts.
  - LN1 mean is folded into the QKV matmul as a rank-1 K=1 accumulation
    (lhsT=mneg row, rhs=u row); rstd applied at PSUM eviction via
    per-partition tensor_scalar (pixels are partitions).
  - Attention output + projection collapsed into per-image G = Wproj @
    concat_h(attn_h @ Wv_h); attn branch = rstd * (G@x - m*uG) via the same
    rank-1 trick; LN2 materializes yn (bf16) for the FFN.
  - Eviction work split across DVE and Activation engines; emission is
    software-pipelined (S-accum deferred one chunk; image-1 attention block
    interleaved into image-0 phase B) so the PE stays fed.
"""

import numpy as np

B, C, NH, CH, N, HID = 16, 384, 8, 48, 4096, 1536
NCORES = 8
BPC = B // NCORES  # images per core
P = 128
KS = C // P   # 3 k-subtiles for C
KH = HID // P  # 12 k-subtiles for HID
NT = N // P   # 32 pixel chunks (phase A)
FG = 512      # phase B pixel chunk
NFG = N // FG
LOGIT_MAX = float(np.log(1.0 / 0.01))
EPS_LN = 1e-5
EPS_NORM = 1e-12

_CACHE = {}


def _patch_tile_drain():
    """Walrus in this env rejects >1 sync-wait on the kernel-tail Drain
    (CTRL_NO_STRUCT setupSyncWait).  Split the waits across a chain of
    drain instructions, one wait each.  Idempotent, in-process only."""
    import concourse.tile as tile
    from concourse import mybir
    from concourse.vector_clock import ScopedClock

    if getattr(tile.TileContext._drain_and_barrier, "_split_patch", False):
        return

    def _split_drain(self, tick_clock, wait_clock):
        drain_inst = self.nc.sync.drain()
        wait_clock.add_sem_waits(
            drain_inst.ins, ScopedClock({None: tick_clock.global_clock}))
        si = drain_inst.ins.sync_info
        if si is not None and si.on_wait and len(si.on_wait) > 1:
            waits = list(si.on_wait)
            si.on_wait = waits[:1]
            for w in waits[1:]:
                d2 = self.nc.sync.drain()
                d2.ins.sync_info = mybir.SyncInfo(on_wait=[w], on_update=[])
        self.nc.all_engine_barrier()
        popped = self.nc._tile_sem_poison_stack.pop()
        assert popped is self._sem_poison
        self.nc.clear_and_free_semaphores(list(self.sems.allocated().values()))
        self.nc.all_engine_barrier()

    _split_drain._split_patch = True
    tile.TileContext._drain_and_barrier = _split_drain


def _split_waits(nc, max_waits=1):
    """This walrus build rejects instructions carrying more than one sync
    wait ('Too many sync wait commands' / 'ISA wrong length').  Move extra
    waits onto same-engine NoOps inserted immediately before."""
    from concourse import mybir

    n = 0
    for fn in nc.m.functions:
        for blk in fn.blocks:
            out = []
            for inst in blk.instructions:
                si = inst.sync_info
                # custom-DVE InstISA can't carry any sync commands at all
                mw = 0 if isinstance(inst, mybir.InstISA) else max_waits
                if si is not None and si.on_wait and len(si.on_wait) > mw:
                    waits = list(si.on_wait)
                    keep = waits[-mw:] if mw else []
                    for w in waits[:len(waits) - mw]:
                        n += 1
                        nop = mybir.InstNoOp(
                            name=f"I-wsplit-{n}", ins=[], outs=[])
                        nop.engine = inst.engine
                        nop.sync_info = mybir.SyncInfo(
                            on_wait=[w], on_update=[])
                        out.append(nop)
                    si.on_wait = keep
                out.append(inst)
                if (isinstance(inst, mybir.InstISA) and si is not None
                        and si.on_update):
                    n += 1
                    nop = mybir.InstNoOp(name=f"I-usplit-{n}", ins=[], outs=[])
                    nop.engine = inst.engine
                    nop.sync_info = mybir.SyncInfo(
                        on_wait=[], on_update=list(si.on_update))
                    out.append(nop)
                    si.on_update = []
            blk.instructions = out
    return nc


def _build_nc():
    import concourse.bass as bass
    import concourse.tile as tile
    from concourse import mybir

    dt = mybir.dt
    AF = mybir.ActivationFunctionType
    ALU = mybir.AluOpType
    AX = mybir.AxisListType
    from concourse.masks import make_identity

    f32 = dt.float32
    bf16 = dt.bfloat16

    _patch_tile_drain()
    nc = bass.Bass()

    xs = nc.declare_dram_parameter("xs", [BPC, C, N], bf16, isOutput=False)
    wqk_t = nc.declare_dram_parameter("wqk_t", [C, 2 * C], bf16, isOutput=False)
    u_qk = nc.declare_dram_parameter("u_qk", [1, 2 * C], bf16, isOutput=False)
    wv = nc.declare_dram_parameter("wv", [CH, NH, C], bf16, isOutput=False)
    wpj48 = nc.declare_dram_parameter("wpj48", [CH, NH, C], bf16, isOutput=False)
    f8 = dt.float8e4
    w1_t = nc.declare_dram_parameter("w1_t", [C, HID], f8, isOutput=False)
    w2_t = nc.declare_dram_parameter("w2_t", [HID, C], f8, isOutput=False)
    scale_row = nc.declare_dram_parameter("scale_row", [1, NH], f32, isOutput=False)
    out_d = nc.declare_dram_parameter("out", [BPC, C, N], bf16, isOutput=True)

    with tile.TileContext(nc) as tc:
        with (
            tc.tile_pool(name="consts", bufs=1) as consts,
            tc.tile_pool(name="ximg", bufs=2) as xpool,
            tc.tile_pool(name="qkp", bufs=2) as qkpool,
            tc.tile_pool(name="attn", bufs=2) as apool,
            tc.tile_pool(name="scr", bufs=3) as scr,
            tc.tile_pool(name="bwork", bufs=2) as bw,
            tc.tile_pool(name="pb", bufs=7, space="PSUM") as ps,
            tc.tile_pool(name="acc", bufs=1, space="PSUM") as psacc,
        ):
            def bcast_read(dst, dram_row, parts):
                src = bass.AP(
                    tensor=dram_row.tensor, offset=dram_row.offset,
                    ap=[[0, parts]] + [list(d) for d in dram_row.ap[-1:]])
                nc.gpsimd.dma_start(dst, src)

            # ---------------- constants ----------------
            # emission order = gpsimd-queue order: small tiles and the
            # weights needed by phase-A chunk 0 first; wv/wpj (attention
            # block) and w1/w2 (phase B2) are not needed for ~100us.
            ones_col = consts.tile([P, 1], bf16, tag="onescol")
            nc.vector.memset(ones_col[:], 1.0)
            ones_row = consts.tile([1, P], bf16, tag="onesrow")
            nc.vector.memset(ones_row[:], 1.0)
            identb = consts.tile([P, P], bf16, tag="identb")
            make_identity(nc, identb[:])
            schb = consts.tile([CH, NH], f32, tag="schb")
            bcast_read(schb[:], scale_row[0, :], parts=CH)
            wqk_sb = consts.tile([P, KS, 2 * C], bf16, tag="wqk")
            nc.gpsimd.dma_start(wqk_sb[:], wqk_t.rearrange("(s p) f -> p s f", p=P))
            uqk_sb = consts.tile([1, 2 * C], bf16, tag="uqk")
            nc.gpsimd.dma_start(uqk_sb[:], u_qk[:])
            wv_sb = consts.tile([CH, NH, C], bf16, tag="wv")
            nc.gpsimd.dma_start(wv_sb[:], wv[:])
            wpj_sb = consts.tile([CH, NH, C], bf16, tag="wpj")
            nc.gpsimd.dma_start(wpj_sb[:], wpj48[:])
            w1_sb = consts.tile([P, KS, HID], f8, tag="w1")
            nc.gpsimd.dma_start(w1_sb[:], w1_t.rearrange("(s p) f -> p s f", p=P))
            w2_sb = consts.tile([P, KH, C], f8, tag="w2")
            nc.gpsimd.dma_start(w2_sb[:], w2_t.rearrange("(s p) f -> p s f", p=P))

            xs_r = xs.rearrange("b (s p) n -> b p s n", p=P)
            out_r = out_d.rearrange("b (s p) n -> b p s n", p=P)

            # ------------- load both images upfront -------------
            x_tiles, rowpairs = [], []
            for img in range(BPC):
                x_sb = xpool.tile([P, KS, N], bf16, tag="x")
                for i in range(8):
                    sl = slice(i * 512, (i + 1) * 512)
                    nc.sync.dma_start(x_sb[:, :, sl], xs_r[img][:, :, sl])
                x_tiles.append(x_sb)
                # LN1 per-pixel rows: -mean and rstd (partition 0)
                mrow = xpool.tile([1, N], bf16, tag="mrow")
                rrow = xpool.tile([1, N], bf16, tag="rrow")
                rowpairs.append((mrow, rrow))

            def alloc_acc():
                # one PSUM bank: S [0:48, 0:384] | q-norms² [0:48, 384:392]
                # | k-norms² row parked at partition 64 [64:65, 0:384]
                acc = psacc.tile([P, 504], f32, tag="acc1")
                return acc

            def phase_a(img, acc, interleave=()):
                """LN1 stats + qkT + S/norm accumulation for one image.
                Stats run one chunk ahead of qkT so the PE never waits on
                the stats DVE chain; S-accum is deferred one chunk behind."""
                x_sb = x_tiles[img]
                mrow, rrow = rowpairs[img]

                def stats_mm2(t):
                    """Batched LN1 stats for chunks t and t+1."""
                    sl = slice(t * P, (t + 2) * P)
                    xsq = scr.tile([P, KS, 2 * P], bf16, tag="xsq", bufs=2)
                    nc.vector.tensor_mul(xsq[:], x_sb[:, :, sl], x_sb[:, :, sl])
                    pstat = ps.tile([P, 2, 2], f32, tag="pb")
                    for cp in range(2):
                        csl = slice((t + cp) * P, (t + cp + 1) * P)
                        for s in range(KS):
                            nc.tensor.matmul(
                                pstat[:, cp, 0:1], x_sb[:, s, csl], ones_col[:],
                                start=(s == 0), stop=(s == KS - 1))
                        for s in range(KS):
                            nc.tensor.matmul(
                                pstat[:, cp, 1:2],
                                xsq[:, s, cp * P:(cp + 1) * P], ones_col[:],
                                start=(s == 0), stop=(s == KS - 1))
                    stat2 = scr.tile([P, 2, 33], bf16, tag="stat2", bufs=2)
                    vcol = scr.tile([P, 2], f32, tag="vcol")
                    msq = scr.tile([P, 2], f32, tag="msq")
                    rcol = scr.tile([P, 2], f32, tag="rcol", bufs=2)
                    nc.scalar.activation(
                        stat2[:, :, 0], pstat[:, :, 0], AF.Copy, scale=-1.0 / C)
                    nc.vector.tensor_scalar(
                        vcol[:], pstat[:, :, 1], 1.0 / C, EPS_LN,
                        op0=ALU.mult, op1=ALU.add)
                    nc.scalar.activation(msq[:], stat2[:, :, 0], AF.Square)
                    nc.vector.tensor_sub(vcol[:], vcol[:], msq[:])
                    nc.scalar.activation(rcol[:], vcol[:], AF.Sqrt)
                    nc.vector.reciprocal(rcol[:], rcol[:])
                    nc.vector.tensor_copy(stat2[:, :, 32], rcol[:])
                    return stat2, rcol

                def stats_tr(t, stat2, cp):
                    sl = slice((t + cp) * P, (t + cp + 1) * P)
                    ptr = ps.tile([33, P], bf16, tag="pb")
                    nc.tensor.transpose(ptr[:], stat2[:, cp, :], identb[:])
                    nc.scalar.copy(mrow[0:1, sl], ptr[0:1, :])
                    nc.scalar.copy(rrow[0:1, sl], ptr[32:33, :])

                pend = None
                cur = stats_mm2(0)
                stats_tr(0, cur[0], 0)
                stats_tr(0, cur[0], 1)
                nxt = None
                for t in range(NT):
                    if 1 <= t <= len(interleave):
                        interleave[t - 1]()
                    sl = slice(t * P, (t + 1) * P)
                    rcol = cur[1][:, t % 2:t % 2 + 1]
                    if t % 2 == 0 and t + 2 < NT:
                        nxt = stats_mm2(t + 2)
                    # qkT x-part into PSUM (two banks)
                    pa1 = ps.tile([P, 512], f32, tag="pb")
                    pa2 = ps.tile([P, 256], f32, tag="pb")
                    for s in range(KS):
                        nc.tensor.matmul(
                            pa1[:], x_sb[:, s, sl], wqk_sb[:, s, 0:512],
                            start=(s == 0), stop=False)
                    for s in range(KS):
                        nc.tensor.matmul(
                            pa2[:], x_sb[:, s, sl], wqk_sb[:, s, 512:768],
                            start=(s == 0), stop=False)
                    if t % 2 == 0 and t + 2 < NT:
                        stats_tr(t + 2, nxt[0], 0)
                        stats_tr(t + 2, nxt[0], 1)
                    # rank-1 mean completion (rows for chunk t are ready)
                    nc.tensor.matmul(
                        pa1[:], mrow[0:1, sl], uqk_sb[0:1, 0:512],
                        start=False, stop=True)
                    nc.tensor.matmul(
                        pa2[:], mrow[0:1, sl], uqk_sb[0:1, 512:768],
                        start=False, stop=True)
                    # deferred S/norm accumulation from previous chunk
                    if pend is not None:
                        _emit_s(acc, *pend)
                    # evictions: qk = rstd*pa (DVE + ACT), qksq = qk² (DVE)
                    qk = qkpool.tile([P, 2 * C], bf16, tag="qk")
                    qksq = qkpool.tile([P, 2 * C], bf16, tag="qksq")
                    nc.vector.tensor_scalar_mul(qk[:, 0:512], pa1[:], rcol)
                    nc.scalar.activation(
                        qk[:, 512:768], pa2[:], AF.Copy, scale=rcol)
                    nc.vector.tensor_mul(qksq[:], qk[:], qk[:])
                    pend = (qk, qksq, t)
                    if t % 2 == 1:
                        cur = nxt
                _emit_s(acc, *pend)

            def _emit_s(acc, qk, qksq, t):
                st, sp = (t == 0), (t == NT - 1)
                for h in range(NH):
                    o = h * 2 * CH
                    nc.tensor.matmul(
                        acc[0:CH, h * CH:(h + 1) * CH],
                        qk[:, o:o + CH], qk[:, o + CH:o + 2 * CH],
                        start=st, stop=sp)
                for h in range(NH):
                    o = h * 2 * CH
                    nc.tensor.matmul(
                        acc[0:CH, 384 + h:385 + h],
                        qksq[:, o:o + CH], ones_col[:],
                        start=st, stop=sp)
                ksq = qksq.rearrange("p (h two c) -> p h two c", two=2, c=CH)
                nc.tensor.matmul(
                    acc[64:65, 0:C], ones_col[:], ksq[:, :, 1, :],
                    start=st, stop=sp)

            def attn_stages(img, acc):
                """Softmax + G build as a list of emission closures."""
                st = {}

                def s0():  # norms + scaled S + softmax -> sSb (bf16)
                    rq = apool.tile([CH, NH], f32, tag="rq", bufs=1)
                    nc.scalar.activation(rq[:], acc[0:CH, 384:392], AF.Sqrt)
                    nc.vector.tensor_scalar_max(rq[:], rq[:], EPS_NORM)
                    nc.vector.reciprocal(rq[:], rq[:])
                    nc.vector.tensor_mul(rq[:], rq[:], schb[:])
                    rk = apool.tile([1, C], f32, tag="rk", bufs=1)
                    nc.scalar.activation(rk[:], acc[64:65, 0:C], AF.Sqrt)
                    nc.vector.tensor_scalar_max(rk[:], rk[:], EPS_NORM)
                    nc.vector.reciprocal(rk[:], rk[:])
                    rkb = apool.tile([1, C], bf16, tag="rkb", bufs=1)
                    nc.vector.tensor_copy(rkb[:], rk[:])
                    rkb_ps = ps.tile([CH, C], f32, tag="pb")
                    nc.tensor.matmul(
                        rkb_ps[:], ones_row[0:1, 0:CH], rkb[0:1, :],
                        start=True, stop=True)
                    sS = apool.tile([CH, NH, CH], f32, tag="sS", bufs=1)
                    s_v = acc[0:CH, 0:384].rearrange("p (h e) -> p h e", e=CH)
                    nc.vector.tensor_mul(
                        sS[:], s_v, rq[:, :, None].to_broadcast((CH, NH, CH)))
                    rkb_v = rkb_ps.rearrange("p (h e) -> p h e", e=CH)
                    nc.vector.tensor_mul(sS[:], sS[:], rkb_v)
                    mx = apool.tile([CH, NH], f32, tag="mx", bufs=1)
                    nc.vector.reduce_max(mx[:], sS[:], axis=AX.X)
                    nc.vector.tensor_sub(
                        sS[:], sS[:], mx[:, :, None].to_broadcast((CH, NH, CH)))
                    nc.scalar.activation(sS[:], sS[:], AF.Exp)
                    esum = apool.tile([CH, NH], f32, tag="esum", bufs=1)
                    nc.vector.reduce_sum(esum[:], sS[:], axis=AX.X)
                    nc.vector.reciprocal(esum[:], esum[:])
                    sSb = apool.tile([CH, NH, CH], bf16, tag="sSb", bufs=1)
                    nc.vector.tensor_mul(
                        sSb[:], sS[:],
                        esum[:, :, None].to_broadcast((CH, NH, CH)))
                    st["sSb"] = sSb

                def s1():  # transpose attn per head
                    pt8 = ps.tile([CH, NH, CH], bf16, tag="pb")
                    for h in range(NH):
                        nc.tensor.transpose(
                            pt8[:, h, :], st["sSb"][:, h, :], identb[0:CH, 0:CH])
                    atT = apool.tile([CH, NH, CH], bf16, tag="atT", bufs=1)
                    nc.vector.tensor_copy(atT[:], pt8[:])
                    st["atT"] = atT

                def s2():  # awv_h = attn_h @ Wv_h
                    awv = apool.tile([CH, NH, C], bf16, tag="awv", bufs=1)
                    for h in range(NH):
                        paw = ps.tile([CH, C], f32, tag="pb")
                        nc.tensor.matmul(
                            paw[:], st["atT"][:, h, :], wv_sb[:, h, :],
                            start=True, stop=True)
                        if h % 2 == 0:
                            nc.vector.tensor_copy(awv[:, h, :], paw[:])
                        else:
                            nc.scalar.copy(awv[:, h, :], paw[:])
                    st["awv"] = awv

                def s3():  # G^T
                    gt_sb = apool.tile([P, KS, C], bf16, tag="gt")
                    for j in range(KS):
                        pgt = ps.tile([P, C], f32, tag="pb")
                        for h in range(NH):
                            nc.tensor.matmul(
                                pgt[:], st["awv"][:, h, j * P:(j + 1) * P],
                                wpj_sb[:, h, :], start=(h == 0), stop=(h == NH - 1))
                        if j % 2 == 0:
                            nc.vector.tensor_copy(gt_sb[:, j, :], pgt[:])
                        else:
                            nc.scalar.copy(gt_sb[:, j, :], pgt[:])
                    st["gt"] = gt_sb

                def s4():  # uG row
                    pug = ps.tile([1, C], f32, tag="pb")
                    for s in range(KS):
                        nc.tensor.matmul(
                            pug[:], ones_col[:], st["gt"][:, s, :],
                            start=(s == 0), stop=(s == KS - 1))
                    ug = apool.tile([1, C], bf16, tag="ug")
                    nc.vector.tensor_copy(ug[:], pug[:])
                    st["ug"] = ug

                return [s0, s1, s2, s3, s4], st

            # per-image y / yn tiles (yn in fp8 for the DR ffn)
            f8sc = 64.0  # host scales w1/w2 by 64 (fp8 e4m3 denormal floor)

            def phase_b1_gen(img, st, y, yn, interleave=()):
                """G-branch apply + residual + LN2; fills y (bf16) and
                yn (fp8) for the whole image. Yields after each chunk so the
                scheduler can interleave other work."""
                mrow, rrow = rowpairs[img]
                gt, ug = st["gt"], st["ug"]
                pends, pend2 = [], []
                for f in range(NFG):
                    sl = slice(f * FG, (f + 1) * FG)
                    xb = scr.tile([P, KS, FG], bf16, tag="xb", bufs=2)
                    nc.sync.dma_start(xb[:], xs_r[img][:, :, sl])
                    bc1 = ps.tile([P, FG], f32, tag="pb")
                    nc.tensor.matmul(
                        bc1[:], ones_row[0:1, :], rrow[0:1, sl],
                        start=True, stop=True)
                    rb = scr.tile([P, FG], bf16, tag="rb", bufs=2)
                    nc.scalar.copy(rb[:], bc1[:])
                    pgs = []
                    for j in range(KS):
                        pg = ps.tile([P, FG], f32, tag="pb", name=f"pg{j}")
                        for s in range(KS):
                            nc.tensor.matmul(
                                pg[:], gt[:, s, j * P:(j + 1) * P],
                                xb[:, s, :], start=(s == 0), stop=False)
                        nc.tensor.matmul(
                            pg[:], ug[0:1, j * P:(j + 1) * P], mrow[0:1, sl],
                            start=False, stop=True)
                        pgs.append(pg)
                    ab = bw.tile([P, KS, FG], bf16, tag="ab", bufs=1)
                    for j in range(KS):
                        nc.vector.tensor_mul(ab[:, j, :], pgs[j][:], rb[:])
                    nc.vector.tensor_add(y[:, :, sl], xb[:], ab[:])
                    ysq = bw.tile([P, KS, FG], bf16, tag="ysq", bufs=2)
                    nc.gpsimd.tensor_mul(ysq[:], y[:, :, sl], y[:, :, sl])
                    # 2-deep pipeline: stats one chunk behind, apply two
                    if f >= 1:
                        pend2.append(_ln2_stats(img, y, f - 1, pends[f - 1]))
                    if f >= 2:
                        _ln2_apply(img, y, yn, f - 2, pend2[f - 2])
                    # interleaved attention stages go after the chunk's
                    # G matmuls so their serial chains don't block the PE
                    if f < len(interleave):
                        interleave[f]()
                    pends.append(ysq)
                    yield f
                pend2.append(_ln2_stats(img, y, NFG - 1, pends[NFG - 1]))
                _ln2_apply(img, y, yn, NFG - 2, pend2[NFG - 2])
                _ln2_apply(img, y, yn, NFG - 1, pend2[NFG - 1])

            def _ln2_stats(img, y, f, ysq):
                sl = slice(f * FG, (f + 1) * FG)
                p2a = ps.tile([1, FG], f32, tag="pb")
                p2b = ps.tile([1, FG], f32, tag="pb")
                for s in range(KS):
                    nc.tensor.matmul(
                        p2a[:], ones_col[:], y[:, s, sl],
                        start=(s == 0), stop=(s == KS - 1))
                for s in range(KS):
                    nc.tensor.matmul(
                        p2b[:], ones_col[:], ysq[:, s, :],
                        start=(s == 0), stop=(s == KS - 1))
                m2b = scr.tile([1, FG], bf16, tag="m2b", bufs=2)
                nc.scalar.activation(m2b[:], p2a[:], AF.Copy, scale=-1.0 / C)
                vrow = scr.tile([1, FG], f32, tag="vrow", bufs=2)
                nc.vector.tensor_scalar(
                    vrow[:], p2b[:], 1.0 / C, EPS_LN, op0=ALU.mult, op1=ALU.add)
                msq = scr.tile([1, FG], f32, tag="msqr", bufs=2)
                nc.scalar.activation(msq[:], m2b[:], AF.Square)
                nc.vector.tensor_sub(vrow[:], vrow[:], msq[:])
                srow = scr.tile([1, FG], f32, tag="srow", bufs=2)
                nc.scalar.activation(srow[:], vrow[:], AF.Sqrt)
                r2f = scr.tile([1, FG], f32, tag="r2f", bufs=2)
                nc.vector.reciprocal(r2f[:], srow[:])
                r2b = scr.tile([1, FG], bf16, tag="r2b", bufs=2)
                nc.scalar.copy(r2b[:], r2f[:])
                return m2b, r2b

            def _ln2_apply(img, y, yn, f, rows):
                sl = slice(f * FG, (f + 1) * FG)
                m2b, r2b = rows
                bcm = ps.tile([P, FG], f32, tag="pb")
                nc.tensor.matmul(
                    bcm[:], ones_row[0:1, :], m2b[0:1, :], start=True, stop=True)
                bcr = ps.tile([P, FG], f32, tag="pb")
                nc.tensor.matmul(
                    bcr[:], ones_row[0:1, :], r2b[0:1, :], start=True, stop=True)
                mbc = scr.tile([P, FG], bf16, tag="mbc", bufs=2)
                nc.scalar.copy(mbc[:], bcm[:])
                rbc = scr.tile([P, FG], bf16, tag="rbc", bufs=2)
                nc.scalar.copy(rbc[:], bcr[:])
                t3 = bw.tile([P, KS, FG], bf16, tag="t3", bufs=1)
                nc.vector.tensor_add(
                    t3[:], y[:, :, sl], mbc[:, None, :].to_broadcast((P, KS, FG)))
                nc.vector.tensor_mul(
                    t3[:], t3[:], rbc[:, None, :].to_broadcast((P, KS, FG)))
                nc.scalar.copy(yn[:, :, sl], t3[:])

            def phase_b2_chunk(img, y, yn, f):
                """One FFN chunk in fp8 DoubleRow + residual + store."""
                if True:
                    sl = slice(f * FG, (f + 1) * FG)
                    h_sb = bw.tile([P, KH, FG], f8, tag="h", bufs=2)
                    for mh in range(KH):
                        ph = ps.tile([P, FG], f32, tag="pb")
                        nc.tensor.matmul(
                            ph[:], w1_sb[:, 0:2, mh * P:(mh + 1) * P],
                            yn[:, 0:2, sl], start=True, stop=False,
                            perf_mode=mybir.MatmulPerfMode.DoubleRow)
                        nc.tensor.matmul(
                            ph[:], w1_sb[:, 2, mh * P:(mh + 1) * P],
                            yn[:, 2, sl], start=False, stop=True)
                        nc.scalar.activation(
                            h_sb[:, mh, :], ph[:], AF.Gelu, scale=1.0 / f8sc)
                    o_sb = bw.tile([P, KS, FG], bf16, tag="o", bufs=1)
                    for mo in range(KS):
                        po = ps.tile([P, FG], f32, tag="pb")
                        for sp in range(KH // 2):
                            nc.tensor.matmul(
                                po[:], w2_sb[:, 2 * sp:2 * sp + 2,
                                             mo * P:(mo + 1) * P],
                                h_sb[:, 2 * sp:2 * sp + 2, :],
                                start=(sp == 0), stop=(sp == KH // 2 - 1),
                                perf_mode=mybir.MatmulPerfMode.DoubleRow)
                        nc.vector.scalar_tensor_tensor(
                            o_sb[:, mo, :], po[:], 1.0 / f8sc, y[:, mo, sl],
                            op0=ALU.mult, op1=ALU.add)
                    nc.sync.dma_start(out_r[img][:, :, sl], o_sb[:])

            # ----------------- schedule -----------------
            acc0 = alloc_acc()
            phase_a(0, acc0)
            stages0, st0 = attn_stages(0, acc0)
            accB = alloc_acc()
            phase_a(1, accB, interleave=stages0)
            stages1, st1 = attn_stages(1, accB)
            y0 = xpool.tile([P, KS, N], bf16, tag="x", name="y0")
            yn0 = bw.tile([P, KS, N], f8, tag="yn", bufs=1, name="yn0")
            for _ in phase_b1_gen(0, st0, y0, yn0, interleave=stages1):
                pass
            for f in range(NFG):
                phase_b2_chunk(0, y0, yn0, f)
            y1 = xpool.tile([P, KS, N], bf16, tag="x", name="y1")
            yn1 = bw.tile([P, KS, N], f8, tag="yn", bufs=1, name="yn1")
            for _ in phase_b1_gen(1, st1, y1, yn1):
                pass
            for f in range(NFG):
                phase_b2_chunk(1, y1, yn1, f)

    return _split_waits(nc)


def _prep_weights(inputs):
    import ml_dtypes
    bf = ml_dtypes.bfloat16
    f8 = ml_dtypes.float8_e4m3
    w_qkv = np.asarray(inputs["w_qkv"], np.float32)
    g1 = np.asarray(inputs["g1"], np.float32)
    g2 = np.asarray(inputs["g2"], np.float32)
    for name in ("beta1", "beta2", "b_qkv", "b_proj", "b_ffn1", "b_ffn2"):
        assert not np.any(np.asarray(inputs[name])), f"{name} nonzero unsupported"
    wg = w_qkv * g1[None, :]  # fold LN gamma into qkv weights
    wg3 = wg.reshape(NH, 3 * CH, C)
    wq = wg3[:, 0:CH, :]
    wk = wg3[:, CH:2 * CH, :]
    wv_ = wg3[:, 2 * CH:3 * CH, :]
    # qk columns interleaved per head: j = h*96 + (0..47 q | 48..95 k)
    wqk = np.concatenate([wq, wk], axis=1).reshape(2 * C, C)
    wqk_t = np.ascontiguousarray(wqk.T)  # [384, 768]
    u_qk = wqk.sum(axis=1)[None, :]  # [1, 768]
    wv_t = np.ascontiguousarray(wv_.transpose(1, 0, 2))  # [48, NH, 384]
    wpj48 = np.ascontiguousarray(
        np.asarray(inputs["w_proj"], np.float32).T.reshape(NH, CH, C)
        .transpose(1, 0, 2))
    w1g = np.asarray(inputs["w_ffn1"], np.float32) * g2[None, :]
    w1_t = np.ascontiguousarray(w1g.T)  # [384, 1536]
    w2_t = np.ascontiguousarray(np.asarray(inputs["w_ffn2"], np.float32).T)
    ls = np.asarray(inputs["logit_scale"], np.float32).reshape(NH)
    scale_row = np.exp(np.minimum(ls, LOGIT_MAX))[None, :]
    # ffn weights scaled by 64 into fp8 e4m3 (compensated at eviction) to
    # stay clear of the e4m3 denormal floor (2^-6)
    return dict(
        wqk_t=wqk_t.astype(bf), u_qk=np.ascontiguousarray(u_qk).astype(bf),
        wv=wv_t.astype(bf), wpj48=wpj48.astype(bf),
        w1_t=(w1_t * 64.0).astype(f8), w2_t=(w2_t * 64.0).astype(f8),
        scale_row=np.ascontiguousarray(scale_row).astype(np.float32))


def _make_in_maps(inputs):
    import ml_dtypes
    x = np.asarray(inputs["x"], np.float32).reshape(B, C, N).astype(
        ml_dtypes.bfloat16)
    wmap = _prep_weights(inputs)
    in_maps = []
    for c in range(NCORES):
        m = dict(wmap)
        m["xs"] = np.ascontiguousarray(x[c * BPC:(c + 1) * BPC])
        in_maps.append(m)
    return in_maps


def kernel(**inputs):
    from concourse.bass_utils import run_bass_kernel_spmd

    if "nc" not in _CACHE:
        _CACHE["nc"] = _build_nc()
    nc = _CACHE["nc"]
    in_maps = _make_in_maps(inputs)
    res = run_bass_kernel_spmd(nc, in_maps, list(range(NCORES)))
    out = np.concatenate(
        [np.asarray(r["out"], np.float32) for r in res.results], axis=0)
    return out.reshape(B, C, 64, 64)
